# revision 1
# baseline (speedup 1.0000x reference)
"""MultiHeadAttention Trainium2 kernel (8 NeuronCores).

Sharding: core c -> batch b=c//4, head group g=c%4 (4 heads, d_model
slice [256g, 256g+256)). Each core computes q/k/v projections for its
heads (full X input, sliced weights), causal attention, and a partial
output projection y_partial = sdpa_g @ Wo[:, slice].T. Host sums the 4
partials per batch and adds bo.

All matmuls run as float32r (TF32-like: full speed, ~1.5e-4 rel err).
Softmax denominators come free via a ones-row appended to V (M=65
matmuls); normalization uses a DRAM-roundtrip partition broadcast.
Causal structure skips fully-masked key blocks (~40% of attention
flops); diagonal blocks use precomputed 0/1 multiplicative masks
applied after exp.
"""
import sys
import os

sys.path.insert(0, "/opt/trn_rl_repo")

import numpy as np

H = 16
D = 1024
DK = 64
B, S = 2, 2048
P = 128
SC = 512           # sequence chunk (matmul free dim)
NSC = S // SC      # 4
NKC = D // P       # 8 contraction chunks for projections
FL = 256           # local features per core (4 heads x 64)
NFC = FL // P      # 2
HL = 4             # local heads
NJB = S // P       # 16 key blocks

_state = {}

# Results of the last kernel() call (for test harness inspection)
last_results = None


def _build_nc():
    import concourse.bass as bass
    import concourse.mybir as mybir
    import concourse.tile as tile
    from concourse import bacc

    f32 = mybir.dt.float32
    f32r = mybir.dt.float32r
    AF = mybir.ActivationFunctionType
    ts = bass.ts

    nc = bacc.Bacc("TRN2", target_bir_lowering=False, debug=False, num_devices=8)

    # DRAM I/O (per-core shapes; data differs per core)
    f32r_ = mybir.dt.float32r
    xqT = nc.dram_tensor("xqT", [D, S], f32r_, kind="ExternalInput")
    xkT = nc.dram_tensor("xkT", [D, S], f32r_, kind="ExternalInput")
    xvT = nc.dram_tensor("xvT", [D, S], f32r_, kind="ExternalInput")
    wqT = nc.dram_tensor("wqT", [D, FL], f32r_, kind="ExternalInput")
    wkT = nc.dram_tensor("wkT", [D, FL], f32r_, kind="ExternalInput")
    wvT = nc.dram_tensor("wvT", [D, FL], f32r_, kind="ExternalInput")
    woT = nc.dram_tensor("woT", [FL, D], f32r_, kind="ExternalInput")
    bqs = nc.dram_tensor("bqs", [P, NFC], f32, kind="ExternalInput")
    bks = nc.dram_tensor("bks", [P, NFC], f32, kind="ExternalInput")
    bvb = nc.dram_tensor("bvb", [P, FL], f32, kind="ExternalInput")
    msk = nc.dram_tensor("msk", [P, 4, SC], mybir.dt.float32r, kind="ExternalInput")
    onec = nc.dram_tensor("onec", [P, NJB * HL], mybir.dt.float32r,
                          kind="ExternalInput")
    yT = nc.dram_tensor("yT", [D, S], f32, kind="ExternalOutput")
    scr = nc.dram_tensor("scr", [HL * NSC, SC], f32, kind="Internal")

    xq_r = xqT.ap().rearrange("(o p) s -> p o s", p=P)
    xk_r = xkT.ap().rearrange("(o p) s -> p o s", p=P)
    xv_r = xvT.ap().rearrange("(o p) s -> p o s", p=P)
    wq_r = wqT.ap().rearrange("(o p) f -> p o f", p=P)
    wk_r = wkT.ap().rearrange("(o p) f -> p o f", p=P)
    wv_r = wvT.ap().rearrange("(o p) f -> p o f", p=P)
    wo_r = woT.ap().rearrange("(o p) d -> p o d", p=P)
    yT_r = yT.ap().rearrange("(o p) s -> p o s", p=P)

    with tile.TileContext(nc) as tc:
        with tc.tile_pool(name="const", bufs=1) as const, \
             tc.tile_pool(name="xpool", bufs=3) as xpool, \
             tc.tile_pool(name="big", bufs=1) as big, \
             tc.tile_pool(name="work", bufs=6) as work, \
             tc.tile_pool(name="zpool", bufs=6) as zpool, \
             tc.tile_pool(name="ytile", bufs=2) as ytile, \
             tc.tile_pool(name="pp", bufs=2, space="PSUM") as pp, \
             tc.tile_pool(name="pss", bufs=3, space="PSUM") as pss, \
             tc.tile_pool(name="pso", bufs=3, space="PSUM") as pso:

            # ---- constants (weights cast to f32r via gpsimd cast-DMA) ----
            w_q = const.tile([P, NKC, FL], f32r, tag="wq")
            w_k = const.tile([P, NKC, FL], f32r, tag="wk")
            w_v = const.tile([P, NKC, FL], f32r, tag="wv")
            w_o = const.tile([P, NFC, D], f32r, tag="wo")
            nc.sync.dma_start(w_q[:], wq_r)
            nc.sync.dma_start(w_k[:], wk_r)
            nc.sync.dma_start(w_v[:], wv_r)
            nc.sync.dma_start(w_o[:], wo_r)
            b_q = const.tile([P, NFC], f32, tag="bq")
            b_k = const.tile([P, NFC], f32, tag="bk")
            b_v = const.tile([P, FL], f32, tag="bv")
            nc.sync.dma_start(b_q[:], bqs.ap())
            nc.sync.dma_start(b_k[:], bks.ap())
            nc.sync.dma_start(b_v[:], bvb.ap())
            masks = const.tile([P, 4, SC], f32r, tag="msk")
            nc.sync.dma_start(masks[:], msk.ap())

            # ---- persistent intermediates ----
            kT = big.tile([P, NFC, S], f32r, tag="kT")
            qT = big.tile([P, NFC, S], f32r, tag="qT")
            vaug = big.tile([P, NJB, HL * (DK + 1)], f32r, tag="vaug")
            sdpaT = big.tile([P, NFC, S], f32r, tag="sdpaT")
            # ones column per head at position 64 within each 65-wide group
            ones_dst = vaug[:].rearrange("p j (h u) -> p j h u", u=DK + 1)[
                :, :, :, DK
            ]
            nc.sync.dma_start(ones_dst, onec.ap().rearrange("p (j h) -> p j h", h=HL))

            # deferred normalization + output projection (one-chunk lag)
            pend = {}

            def _normalize_and_wo(cc):
                for h in range(HL):
                    z, bc = pend.pop((cc, h))
                    base = 64 * (h % 2)
                    fc = h // 2
                    nc.vector.tensor_tensor(
                        sdpaT[base : base + DK, fc, ts(cc, SC)], z[:], bc[:],
                        bass.mybir.AluOpType.mult,
                    )
                for mo in range(D // P):
                    ps_y = pss.tile([P, SC], f32, tag="ps", name=f"py_{cc}_{mo}")
                    for fc in range(NFC):
                        nc.tensor.matmul(
                            ps_y[:], w_o[:, fc, ts(mo, P)],
                            sdpaT[:, fc, ts(cc, SC)],
                            start=(fc == 0), stop=(fc == NFC - 1),
                            skip_group_check=True,
                        )
                    ys = ytile.tile([P, SC], f32, tag="y", name=f"ys_{cc}_{mo}")
                    nc.vector.tensor_copy(ys[:], ps_y[:])
                    nc.sync.dma_start(yT_r[:, mo, ts(cc, SC)], ys[:])

            # ---- per sequence-chunk: K/Q/V projections, attention, Wo ----
            for c in range(NSC):
                # K projection for chunk c
                for name, x_r, w_t, b_t, outT in (
                    ("k", xk_r, w_k, b_k, kT),
                    ("q", xq_r, w_q, b_q, qT),
                ):
                    xt = xpool.tile([P, NKC, SC], f32r, tag="x",
                                    name=f"x{name}_{c}")
                    nc.sync.dma_start(xt[:], x_r[:, :, ts(c, SC)])
                    for fc in range(NFC):
                        ps = pp.tile([P, SC], f32, tag="p512",
                                     name=f"pp{name}_{c}_{fc}")
                        for k in range(NKC):
                            nc.tensor.matmul(
                                ps[:], w_t[:, k, ts(fc, P)], xt[:, k, :],
                                start=(k == 0), stop=(k == NKC - 1),
                            )
                        nc.vector.tensor_scalar_add(
                            outT[:, fc, ts(c, SC)], ps[:], b_t[:, fc : fc + 1]
                        )
                # V projection for chunk c
                xt = xpool.tile([P, NKC, SC], f32r, tag="x", name=f"xv_{c}")
                nc.sync.dma_start(xt[:], xv_r[:, :, ts(c, SC)])
                for sb in range(SC // P):  # 4 s-blocks of 128 per chunk
                    j = c * 4 + sb
                    ps = pp.tile([P, SC], f32, tag="p512", name=f"ppv_{j}")
                    for k in range(NKC):
                        nc.tensor.matmul(
                            ps[:, :FL], xt[:, k, ts(sb, P)], w_v[:, k, :],
                            start=(k == 0), stop=(k == NKC - 1),
                        )
                    dst = vaug[:, j].rearrange("p (h u) -> p h u", u=DK + 1)[:, :, :DK]
                    src = ps[:, :FL].rearrange("p (h u) -> p h u", u=DK)
                    bsrc = b_v[:].rearrange("p (h u) -> p h u", u=DK)
                    nc.vector.tensor_tensor(
                        dst, src, bsrc, bass.mybir.AluOpType.add
                    )

                # attention for chunk c (keys j <= 4c+3 are all projected)
                n_j = 4 * c + 4
                for hp in range(2):  # head pairs (0,1) fc=0 and (2,3) fc=1
                    heads = (2 * hp, 2 * hp + 1)
                    ps_o = {}
                    for h in heads:
                        ps_o[h] = pso.tile(
                            [DK + 1, SC], f32, tag="po", name=f"po_{c}_{h}"
                        )
                    ets = {}
                    j_order = list(range(4 * c, n_j)) + list(range(4 * c))
                    for jx, j in enumerate(j_order):
                        for h in heads:
                            base = 64 * (h % 2)
                            fc = h // 2
                            ps_s = pss.tile([P, SC], f32, tag="ps")
                            nc.tensor.matmul(
                                ps_s[:],
                                kT[base : base + DK, fc, ts(j, P)],
                                qT[base : base + DK, fc, ts(c, SC)],
                                start=True, stop=True, skip_group_check=True,
                            )
                            et = work.tile([P, SC], f32r, tag="et")
                            nc.scalar.activation(et[:], ps_s[:], AF.Exp)
                            if j >= 4 * c:
                                nc.vector.tensor_tensor(
                                    et[:], et[:], masks[:, j - 4 * c, :],
                                    bass.mybir.AluOpType.mult,
                                )
                            ets[h] = et
                        for h in heads:
                            nc.tensor.matmul(
                                ps_o[h][:],
                                vaug[:, j, (DK + 1) * h : (DK + 1) * (h + 1)],
                                ets[h][:],
                                start=(jx == 0), stop=(jx == n_j - 1),
                                skip_group_check=True,
                            )
                    # evict unnormalized sdpa + launch the recip roundtrip;
                    # normalization + Wo happen one chunk later (sw pipeline)
                    for h in heads:
                        z = zpool.tile([DK, SC], f32, tag="z", name=f"z_{c}_{h}")
                        nc.vector.tensor_copy(z[:], ps_o[h][:DK, :])
                        rc = work.tile([1, SC], f32, tag="rc", name=f"rc_{c}_{h}")
                        nc.vector.reciprocal(rc[:], ps_o[h][DK : DK + 1, :])
                        idx = h * NSC + c
                        nc.sync.dma_start(scr.ap()[idx : idx + 1, :], rc[:])
                        bc = zpool.tile([DK, SC], f32, tag="bc", name=f"bc_{c}_{h}")
                        nc.sync.dma_start(
                            bc[:], scr.ap()[idx : idx + 1, :].to_broadcast((DK, SC))
                        )
                        pend[(c, h)] = (z, bc)

                if c > 0:
                    _normalize_and_wo(c - 1)
            _normalize_and_wo(NSC - 1)

    nc.compile()
    return nc


def _get_nc():
    if "nc" not in _state:
        _state["nc"] = _build_nc()
    return _state["nc"]


def kernel(Q, K, V, mask, Wq, bq, Wk, bk, Wv, bv, Wo, bo):
    global last_results
    from concourse.bass_utils import run_bass_kernel_spmd

    Q = np.asarray(Q, np.float32)
    K = np.asarray(K, np.float32)
    V = np.asarray(V, np.float32)
    Wq = np.asarray(Wq, np.float32)
    bq = np.asarray(bq, np.float32)
    Wk = np.asarray(Wk, np.float32)
    bk = np.asarray(bk, np.float32)
    Wv = np.asarray(Wv, np.float32)
    bv = np.asarray(bv, np.float32)
    Wo = np.asarray(Wo, np.float32)
    bo = np.asarray(bo, np.float32)

    nc = _get_nc()

    # causal 0/1 masks for the 4 diagonal block offsets
    p = np.arange(P)[:, None, None]
    m = np.arange(4)[None, :, None]
    x = np.arange(SC)[None, None, :]
    msk_np = (x >= P * m + p).astype(np.float32)

    xT = {}
    for b in range(B):
        xT[("q", b)] = np.ascontiguousarray(Q[b].T)
        xT[("k", b)] = np.ascontiguousarray(K[b].T)
        xT[("v", b)] = np.ascontiguousarray(V[b].T)

    in_maps = []
    for core in range(8):
        b = core // 4
        g = core % 4
        fs, fe = FL * g, FL * (g + 1)
        # fold the 1/sqrt(dk)=0.125 score scale into the q side (exact)
        wq_s = np.ascontiguousarray((Wq[fs:fe, :] * 0.125).T)
        bq_s = bq[fs:fe] * 0.125
        in_maps.append({
            "xqT": xT[("q", b)],
            "xkT": xT[("k", b)],
            "xvT": xT[("v", b)],
            "wqT": wq_s,
            "wkT": np.ascontiguousarray(Wk[fs:fe, :].T),
            "wvT": np.ascontiguousarray(Wv[fs:fe, :].T),
            "woT": np.ascontiguousarray(Wo[:, fs:fe].T),
            "bqs": np.ascontiguousarray(bq_s.reshape(NFC, P).T),
            "bks": np.ascontiguousarray(bk[fs:fe].reshape(NFC, P).T),
            "bvb": np.ascontiguousarray(
                np.broadcast_to(bv[fs:fe][None, :], (P, FL))
            ),
            "msk": msk_np,
            "onec": np.ones((P, NJB * HL), np.float32),
        })

    res = run_bass_kernel_spmd(nc, in_maps, core_ids=list(range(8)))
    last_results = res

    out = np.empty((B, S, D), np.float32)
    for b in range(B):
        acc = res.results[4 * b]["yT"].astype(np.float32).copy()
        for g in range(1, 4):
            acc += res.results[4 * b + g]["yT"]
        out[b] = acc.T + bo[None, :]
    return out



# revision 2
# speedup vs baseline: 1.4743x; 1.4743x over previous
"""MultiHeadAttention Trainium2 kernel (8 NeuronCores).

Sharding: core c -> batch b=c//4, head group g=c%4 (4 heads, d_model
slice [256g, 256g+256)). Each core computes q/k/v projections for its
heads (full X input, sliced weights), causal attention, and a partial
output projection y_partial = sdpa_g @ Wo[:, slice].T. Host sums the 4
partials per batch and adds bo.

v2: all matmul operands bf16 (halves DMA + weight-load time, lowers
power). Score tiles for a head PAIR go into one 2-bank PSUM tile so a
single activation computes exp over [128, 2*512] (halves Act-engine
instruction overhead -- Act is the attention-phase bottleneck). AV
matmuls lag scores by one j-block so the PE never waits on exp.
Softmax denominators come free via a ones-row appended to V; the
reciprocal+broadcast is done in-chunk: DVE copy of the denom row,
PE ones-matmul broadcast to 64 partitions, DVE reciprocal_approx_fast,
DVE multiply into sdpaT. Output partials are bf16.
"""
import sys
import os

sys.path.insert(0, "/opt/trn_rl_repo")

import numpy as np

H = 16
D = 1024
DK = 64
B, S = 2, 2048
P = 128
SC = 512           # sequence chunk (matmul free dim)
NSC = S // SC      # 4
NKC = D // P       # 8 contraction chunks for projections
FL = 256           # local features per core (4 heads x 64)
NFC = FL // P      # 2
HL = 4             # local heads
NJB = S // P       # 16 key blocks

_state = {}

# Results of the last kernel() call (for test harness inspection)
last_results = None


def _build_nc():
    import concourse.bass as bass
    import concourse.mybir as mybir
    import concourse.tile as tile
    from concourse import bacc

    f32 = mybir.dt.float32
    bf16 = mybir.dt.bfloat16
    AF = mybir.ActivationFunctionType
    ts = bass.ts

    nc = bacc.Bacc("TRN2", target_bir_lowering=False, debug=False, num_devices=8)

    # DRAM I/O (per-core shapes; data differs per core)
    xqT = nc.dram_tensor("xqT", [D, S], bf16, kind="ExternalInput")
    xkT = nc.dram_tensor("xkT", [D, S], bf16, kind="ExternalInput")
    xvT = nc.dram_tensor("xvT", [D, S], bf16, kind="ExternalInput")
    wqT = nc.dram_tensor("wqT", [D, FL], bf16, kind="ExternalInput")
    wkT = nc.dram_tensor("wkT", [D, FL], bf16, kind="ExternalInput")
    wvT = nc.dram_tensor("wvT", [D, FL], bf16, kind="ExternalInput")
    woT = nc.dram_tensor("woT", [FL, D], bf16, kind="ExternalInput")
    bqs = nc.dram_tensor("bqs", [P, NFC], f32, kind="ExternalInput")
    bks = nc.dram_tensor("bks", [P, NFC], f32, kind="ExternalInput")
    bvb = nc.dram_tensor("bvb", [P, FL], f32, kind="ExternalInput")
    # mask per diagonal offset d, duplicated for the two heads of a pair
    msk = nc.dram_tensor("msk", [P, 4, 2, SC], bf16, kind="ExternalInput")
    yT = nc.dram_tensor("yT", [D, S], bf16, kind="ExternalOutput")

    xq_r = xqT.ap().rearrange("(o p) s -> p o s", p=P)
    xk_r = xkT.ap().rearrange("(o p) s -> p o s", p=P)
    xv_r = xvT.ap().rearrange("(o p) s -> p o s", p=P)
    wq_r = wqT.ap().rearrange("(o p) f -> p o f", p=P)
    wk_r = wkT.ap().rearrange("(o p) f -> p o f", p=P)
    wv_r = wvT.ap().rearrange("(o p) f -> p o f", p=P)
    wo_r = woT.ap().rearrange("(o p) d -> p o d", p=P)
    yT_r = yT.ap().rearrange("(o p) s -> p o s", p=P)

    with tile.TileContext(nc) as tc:
        with tc.tile_pool(name="const", bufs=1) as const, \
             tc.tile_pool(name="xpool", bufs=3) as xpool, \
             tc.tile_pool(name="big", bufs=1) as big, \
             tc.tile_pool(name="etp", bufs=3) as etp, \
             tc.tile_pool(name="small", bufs=3) as small, \
             tc.tile_pool(name="ytile", bufs=2) as ytile, \
             tc.tile_pool(name="pp", bufs=2, space="PSUM") as pp, \
             tc.tile_pool(name="pss", bufs=2, space="PSUM") as pss, \
             tc.tile_pool(name="pso", bufs=2, space="PSUM") as pso:

            # ---- constants ----
            w_q = const.tile([P, NKC, FL], bf16, tag="wq")
            w_k = const.tile([P, NKC, FL], bf16, tag="wk")
            w_v = const.tile([P, NKC, FL], bf16, tag="wv")
            w_o = const.tile([P, NFC, D], bf16, tag="wo")
            nc.sync.dma_start(w_q[:], wq_r)
            nc.sync.dma_start(w_k[:], wk_r)
            nc.sync.dma_start(w_v[:], wv_r)
            nc.sync.dma_start(w_o[:], wo_r)
            b_q = const.tile([P, NFC], f32, tag="bq")
            b_k = const.tile([P, NFC], f32, tag="bk")
            b_v = const.tile([P, FL], f32, tag="bv")
            nc.sync.dma_start(b_q[:], bqs.ap())
            nc.sync.dma_start(b_k[:], bks.ap())
            nc.sync.dma_start(b_v[:], bvb.ap())
            masks = const.tile([P, 4, 2, SC], bf16, tag="msk")
            nc.sync.dma_start(masks[:], msk.ap())
            ones_r = const.tile([1, DK], bf16, tag="ones")
            nc.gpsimd.memset(ones_r[:], 1.0)

            # ---- persistent intermediates ----
            kT = big.tile([P, NFC, S], bf16, tag="kT")
            qT = big.tile([P, NFC, S], bf16, tag="qT")
            vaug = big.tile([P, NJB, HL * (DK + 1)], bf16, tag="vaug")
            sdpaT = big.tile([P, NFC, S], bf16, tag="sdpaT")
            # ones column per head at position 64 within each 65-wide group
            ones_dst = vaug[:].rearrange("p j (h u) -> p j h u", u=DK + 1)[
                :, :, :, DK
            ]
            nc.gpsimd.memset(ones_dst, 1.0)

            # ---- per sequence-chunk: K/Q/V projections, attention, Wo ----
            for c in range(NSC):
                for name, x_r, w_t, b_t, outT in (
                    ("k", xk_r, w_k, b_k, kT),
                    ("q", xq_r, w_q, b_q, qT),
                ):
                    xt = xpool.tile([P, NKC, SC], bf16, tag="x",
                                    name=f"x{name}_{c}")
                    nc.sync.dma_start(xt[:], x_r[:, :, ts(c, SC)])
                    for fc in range(NFC):
                        ps = pp.tile([P, SC], f32, tag="p512",
                                     name=f"pp{name}_{c}_{fc}")
                        for k in range(NKC):
                            nc.tensor.matmul(
                                ps[:], w_t[:, k, ts(fc, P)], xt[:, k, :],
                                start=(k == 0), stop=(k == NKC - 1),
                            )
                        nc.vector.tensor_scalar_add(
                            outT[:, fc, ts(c, SC)], ps[:], b_t[:, fc : fc + 1]
                        )
                # V projection for chunk c
                xt = xpool.tile([P, NKC, SC], bf16, tag="x", name=f"xv_{c}")
                nc.sync.dma_start(xt[:], xv_r[:, :, ts(c, SC)])
                for sb in range(SC // P):  # 4 s-blocks of 128 per chunk
                    j = c * 4 + sb
                    ps = pp.tile([P, SC], f32, tag="p512", name=f"ppv_{j}")
                    for k in range(NKC):
                        nc.tensor.matmul(
                            ps[:, :FL], xt[:, k, ts(sb, P)], w_v[:, k, :],
                            start=(k == 0), stop=(k == NKC - 1),
                        )
                    dst = vaug[:, j].rearrange("p (h u) -> p h u", u=DK + 1)[:, :, :DK]
                    src = ps[:, :FL].rearrange("p (h u) -> p h u", u=DK)
                    bsrc = b_v[:].rearrange("p (h u) -> p h u", u=DK)
                    nc.vector.tensor_tensor(
                        dst, src, bsrc, bass.mybir.AluOpType.add
                    )

                # attention for chunk c (keys j <= 4c+3 are all projected)
                n_j = 4 * c + 4
                for hp in range(2):  # head pairs (0,1) fc=0 and (2,3) fc=1
                    h0, h1 = 2 * hp, 2 * hp + 1
                    ps_o = {}
                    for h in (h0, h1):
                        ps_o[h] = pso.tile(
                            [DK + 1, SC], f32, tag="po", name=f"po_{c}_{h}"
                        )
                    j_order = list(range(4 * c, n_j)) + list(range(4 * c))
                    prev = None
                    for jx, j in enumerate(j_order):
                        ps2 = pss.tile([P, 2, SC], f32, tag="s2",
                                       name=f"s2_{c}_{hp}_{jx}")
                        nc.tensor.matmul(
                            ps2[:, 0, :], kT[0:DK, hp, ts(j, P)],
                            qT[0:DK, hp, ts(c, SC)],
                            start=True, stop=True, skip_group_check=True,
                        )
                        nc.tensor.matmul(
                            ps2[:, 1, :], kT[DK : 2 * DK, hp, ts(j, P)],
                            qT[DK : 2 * DK, hp, ts(c, SC)],
                            start=True, stop=True, skip_group_check=True,
                        )
                        et2 = etp.tile([P, 2, SC], bf16, tag="et",
                                       name=f"et_{c}_{hp}_{jx}")
                        nc.scalar.activation(et2[:], ps2[:], AF.Exp)
                        if j >= 4 * c:
                            nc.vector.tensor_tensor(
                                et2[:], et2[:], masks[:, j - 4 * c, :, :],
                                bass.mybir.AluOpType.mult,
                            )
                        if prev is not None:
                            pj, pet = prev
                            for h in (h0, h1):
                                nc.tensor.matmul(
                                    ps_o[h][:],
                                    vaug[:, pj, (DK + 1) * h : (DK + 1) * (h + 1)],
                                    pet[:, h % 2, :],
                                    start=(jx == 1), stop=False,
                                    skip_group_check=True,
                                )
                        prev = (j, et2)
                    pj, pet = prev
                    for h in (h0, h1):
                        nc.tensor.matmul(
                            ps_o[h][:],
                            vaug[:, pj, (DK + 1) * h : (DK + 1) * (h + 1)],
                            pet[:, h % 2, :],
                            start=(n_j == 1), stop=True,
                            skip_group_check=True,
                        )
                    # softmax normalize: denom row -> broadcast -> recip -> mult
                    for h in (h0, h1):
                        dbf = small.tile([1, SC], bf16, tag="dbf",
                                         name=f"dbf_{c}_{h}")
                        nc.vector.tensor_copy(dbf[:], ps_o[h][DK : DK + 1, :])
                        ps_bc = pp.tile([P, SC], f32, tag="p512",
                                        name=f"bc_{c}_{h}")
                        nc.tensor.matmul(
                            ps_bc[:DK, :], ones_r[:], dbf[:],
                            start=True, stop=True, skip_group_check=True,
                        )
                        rc64 = small.tile([DK, SC], f32, tag="rc",
                                          name=f"rc_{c}_{h}")
                        nc.vector.reciprocal_approx_fast(rc64[:], ps_bc[:DK, :])
                        base = DK * (h % 2)
                        nc.vector.tensor_tensor(
                            sdpaT[base : base + DK, h // 2, ts(c, SC)],
                            ps_o[h][:DK, :], rc64[:],
                            bass.mybir.AluOpType.mult,
                        )

                # output projection for chunk c
                for mo in range(D // P):
                    ps_y = pp.tile([P, SC], f32, tag="p512", name=f"py_{c}_{mo}")
                    for fc in range(NFC):
                        nc.tensor.matmul(
                            ps_y[:], w_o[:, fc, ts(mo, P)],
                            sdpaT[:, fc, ts(c, SC)],
                            start=(fc == 0), stop=(fc == NFC - 1),
                            skip_group_check=True,
                        )
                    ys = ytile.tile([P, SC], bf16, tag="y", name=f"ys_{c}_{mo}")
                    nc.vector.tensor_copy(ys[:], ps_y[:])
                    nc.sync.dma_start(yT_r[:, mo, ts(c, SC)], ys[:])

    nc.compile()
    return nc


def _get_nc():
    if "nc" not in _state:
        _state["nc"] = _build_nc()
    return _state["nc"]


def kernel(Q, K, V, mask, Wq, bq, Wk, bk, Wv, bv, Wo, bo):
    global last_results
    from concourse.bass_utils import run_bass_kernel_spmd
    import ml_dtypes

    bf16 = ml_dtypes.bfloat16

    Q = np.asarray(Q, np.float32)
    K = np.asarray(K, np.float32)
    V = np.asarray(V, np.float32)
    Wq = np.asarray(Wq, np.float32)
    bq = np.asarray(bq, np.float32)
    Wk = np.asarray(Wk, np.float32)
    bk = np.asarray(bk, np.float32)
    Wv = np.asarray(Wv, np.float32)
    bv = np.asarray(bv, np.float32)
    Wo = np.asarray(Wo, np.float32)
    bo = np.asarray(bo, np.float32)

    nc = _get_nc()

    # causal 0/1 masks for the 4 diagonal block offsets, duplicated for
    # both heads of a pair
    p = np.arange(P)[:, None, None]
    m = np.arange(4)[None, :, None]
    x = np.arange(SC)[None, None, :]
    msk_np = (x >= P * m + p).astype(bf16)            # [P, 4, SC]
    msk_np = np.ascontiguousarray(
        np.broadcast_to(msk_np[:, :, None, :], (P, 4, 2, SC))
    )

    xT = {}
    for b in range(B):
        xT[("q", b)] = np.ascontiguousarray(Q[b].T.astype(bf16))
        xT[("k", b)] = np.ascontiguousarray(K[b].T.astype(bf16))
        xT[("v", b)] = np.ascontiguousarray(V[b].T.astype(bf16))

    in_maps = []
    for core in range(8):
        b = core // 4
        g = core % 4
        fs, fe = FL * g, FL * (g + 1)
        # fold the 1/sqrt(dk)=0.125 score scale into the q side (exact)
        wq_s = np.ascontiguousarray((Wq[fs:fe, :] * 0.125).T.astype(bf16))
        bq_s = bq[fs:fe] * 0.125
        in_maps.append({
            "xqT": xT[("q", b)],
            "xkT": xT[("k", b)],
            "xvT": xT[("v", b)],
            "wqT": wq_s,
            "wkT": np.ascontiguousarray(Wk[fs:fe, :].T.astype(bf16)),
            "wvT": np.ascontiguousarray(Wv[fs:fe, :].T.astype(bf16)),
            "woT": np.ascontiguousarray(Wo[:, fs:fe].T.astype(bf16)),
            "bqs": np.ascontiguousarray(bq_s.reshape(NFC, P).T),
            "bks": np.ascontiguousarray(bk[fs:fe].reshape(NFC, P).T),
            "bvb": np.ascontiguousarray(
                np.broadcast_to(bv[fs:fe][None, :], (P, FL))
            ),
            "msk": msk_np,
        })

    res = run_bass_kernel_spmd(nc, in_maps, core_ids=list(range(8)))
    last_results = res

    out = np.empty((B, S, D), np.float32)
    for b in range(B):
        acc = res.results[4 * b]["yT"].astype(np.float32)
        for g in range(1, 4):
            acc = acc + res.results[4 * b + g]["yT"].astype(np.float32)
        out[b] = acc.T + bo[None, :]
    return out


# revision 3
# speedup vs baseline: 2.3765x; 1.6119x over previous
"""MultiHeadAttention Trainium2 kernel (8 NeuronCores).

Sharding: core c -> batch b=c//4, head group g=c%4 (4 heads, d_model
slice [256g, 256g+256)). Each core computes q/k/v projections for its
heads (full X input, sliced weights), causal attention, and a partial
output projection y_partial = sdpa_g @ Wo[:, slice].T. Host sums the 4
partials per batch and adds bo.

v3: all matmul operands bf16. Score tiles for a head PAIR go into one
2-bank PSUM tile so a single activation computes exp over [128, 2*512]
(Act is the attention-phase bottleneck). AV matmuls lag scores by one
j-block. The attention phase is act-latency-bound, so projection
matmuls for chunk c+1 and Wo matmuls for chunk c-1 are interleaved as
filler units into attention(c)'s j-loop to keep the PE busy.
Diagonal blocks: exp runs only on the non-fully-masked column
suffix, the fully-masked prefix is zeroed by a gpsimd memset, and the
causal 0/1 triangle multiply ([128,2,128], gpsimd) replaces the
full-width mask. Softmax: ones-row in V gives denominators; in-chunk
DVE copy -> PE ones-matmul broadcast -> DVE reciprocal_approx_fast ->
DVE multiply into sdpaT. Output partials are bf16, summed on host.
"""
import sys
import os

sys.path.insert(0, "/opt/trn_rl_repo")

import numpy as np

H = 16
D = 1024
DK = 64
B, S = 2, 2048
P = 128
SC = 512           # sequence chunk (matmul free dim)
NSC = S // SC      # 4
NKC = D // P       # 8 contraction chunks for projections
FL = 256           # local features per core (4 heads x 64)
NFC = FL // P      # 2
HL = 4             # local heads
NJB = S // P       # 16 key blocks

_state = {}

# Results of the last kernel() call (for test harness inspection)
last_results = None


def _build_nc():
    import concourse.bass as bass
    import concourse.mybir as mybir
    import concourse.tile as tile
    from concourse import bacc

    f32 = mybir.dt.float32
    bf16 = mybir.dt.bfloat16
    AF = mybir.ActivationFunctionType
    ts = bass.ts

    nc = bacc.Bacc("TRN2", target_bir_lowering=False, debug=False, num_devices=8)

    xqT = nc.dram_tensor("xqT", [D, S], bf16, kind="ExternalInput")
    xkT = nc.dram_tensor("xkT", [D, S], bf16, kind="ExternalInput")
    xvT = nc.dram_tensor("xvT", [D, S], bf16, kind="ExternalInput")
    wqT = nc.dram_tensor("wqT", [D, FL], bf16, kind="ExternalInput")
    wkT = nc.dram_tensor("wkT", [D, FL], bf16, kind="ExternalInput")
    wvT = nc.dram_tensor("wvT", [D, FL], bf16, kind="ExternalInput")
    woT = nc.dram_tensor("woT", [FL, D], bf16, kind="ExternalInput")
    bqs = nc.dram_tensor("bqs", [P, NFC], f32, kind="ExternalInput")
    bks = nc.dram_tensor("bks", [P, NFC], f32, kind="ExternalInput")
    bvb = nc.dram_tensor("bvb", [P, FL], f32, kind="ExternalInput")
    # causal triangle mask [p, x] = (x >= p), duplicated for both heads
    msk = nc.dram_tensor("msk", [P, 2, P], bf16, kind="ExternalInput")
    yT = nc.dram_tensor("yT", [D, S], bf16, kind="ExternalOutput")

    xq_r = xqT.ap().rearrange("(o p) s -> p o s", p=P)
    xk_r = xkT.ap().rearrange("(o p) s -> p o s", p=P)
    xv_r = xvT.ap().rearrange("(o p) s -> p o s", p=P)
    wq_r = wqT.ap().rearrange("(o p) f -> p o f", p=P)
    wk_r = wkT.ap().rearrange("(o p) f -> p o f", p=P)
    wv_r = wvT.ap().rearrange("(o p) f -> p o f", p=P)
    wo_r = woT.ap().rearrange("(o p) d -> p o d", p=P)
    yT_r = yT.ap().rearrange("(o p) s -> p o s", p=P)

    with tile.TileContext(nc) as tc:
        with tc.tile_pool(name="const", bufs=1) as const, \
             tc.tile_pool(name="xpool", bufs=6) as xpool, \
             tc.tile_pool(name="big", bufs=1) as big, \
             tc.tile_pool(name="etp", bufs=3) as etp, \
             tc.tile_pool(name="small", bufs=3) as small, \
             tc.tile_pool(name="ytile", bufs=2) as ytile, \
             tc.tile_pool(name="pp", bufs=2, space="PSUM") as pp, \
             tc.tile_pool(name="pss", bufs=2, space="PSUM") as pss, \
             tc.tile_pool(name="pso", bufs=2, space="PSUM") as pso:

            # ---- constants ----
            w_q = const.tile([P, NKC, FL], bf16, tag="wq")
            w_k = const.tile([P, NKC, FL], bf16, tag="wk")
            w_v = const.tile([P, NKC, FL], bf16, tag="wv")
            w_o = const.tile([P, NFC, D], bf16, tag="wo")
            nc.sync.dma_start(w_q[:], wq_r)
            nc.sync.dma_start(w_k[:], wk_r)
            nc.sync.dma_start(w_v[:], wv_r)
            nc.sync.dma_start(w_o[:], wo_r)
            b_q = const.tile([P, NFC], f32, tag="bq")
            b_k = const.tile([P, NFC], f32, tag="bk")
            b_v = const.tile([P, FL], f32, tag="bv")
            nc.sync.dma_start(b_q[:], bqs.ap())
            nc.sync.dma_start(b_k[:], bks.ap())
            nc.sync.dma_start(b_v[:], bvb.ap())
            masks = const.tile([P, 2, P], bf16, tag="msk")
            nc.sync.dma_start(masks[:], msk.ap())
            ones_r = const.tile([1, DK], bf16, tag="ones")
            nc.gpsimd.memset(ones_r[:], 1.0)

            # ---- persistent intermediates ----
            kT = big.tile([P, NFC, S], bf16, tag="kT")
            qT = big.tile([P, NFC, S], bf16, tag="qT")
            vaug = big.tile([P, NJB, HL * (DK + 1)], bf16, tag="vaug")
            sdpaT = big.tile([P, NFC, S], bf16, tag="sdpaT")
            ones_dst = vaug[:].rearrange("p j (h u) -> p j h u", u=DK + 1)[
                :, :, :, DK
            ]
            nc.gpsimd.memset(ones_dst, 1.0)

            # ---- filler unit builders (each unit: () -> None, emits ops) ----
            def make_proj_fillers(c):
                tiles = {}
                for name, x_r in (("k", xk_r), ("q", xq_r), ("v", xv_r)):
                    xt = xpool.tile([P, NKC, SC], bf16, tag="x",
                                    name=f"x{name}_{c}")
                    nc.sync.dma_start(xt[:], x_r[:, :, ts(c, SC)])
                    tiles[name] = xt
                units = []
                for name, w_t, b_t, outT in (("k", w_k, b_k, kT),
                                             ("q", w_q, b_q, qT)):
                    for fc in range(NFC):
                        def u(name=name, fc=fc, w_t=w_t, b_t=b_t, outT=outT):
                            xt = tiles[name]
                            ps = pp.tile([P, SC], f32, tag="p512",
                                         name=f"pp{name}_{c}_{fc}")
                            for k in range(NKC):
                                nc.tensor.matmul(
                                    ps[:], w_t[:, k, ts(fc, P)], xt[:, k, :],
                                    start=(k == 0), stop=(k == NKC - 1),
                                )
                            nc.vector.tensor_scalar_add(
                                outT[:, fc, ts(c, SC)], ps[:],
                                b_t[:, fc : fc + 1],
                            )
                        units.append(u)
                for sb in range(SC // P):
                    def uv(sb=sb):
                        xt = tiles["v"]
                        j = c * 4 + sb
                        ps = pp.tile([P, SC], f32, tag="p512", name=f"ppv_{j}")
                        for k in range(NKC):
                            nc.tensor.matmul(
                                ps[:, :FL], xt[:, k, ts(sb, P)], w_v[:, k, :],
                                start=(k == 0), stop=(k == NKC - 1),
                            )
                        dst = vaug[:, j].rearrange(
                            "p (h u) -> p h u", u=DK + 1)[:, :, :DK]
                        src = ps[:, :FL].rearrange("p (h u) -> p h u", u=DK)
                        bsrc = b_v[:].rearrange("p (h u) -> p h u", u=DK)
                        nc.vector.tensor_tensor(
                            dst, src, bsrc, bass.mybir.AluOpType.add
                        )
                    units.append(uv)
                return units

            def make_wo_fillers(c):
                units = []
                for mo in range(D // P):
                    def uw(mo=mo):
                        ps_y = pp.tile([P, SC], f32, tag="p512",
                                       name=f"py_{c}_{mo}")
                        for fc in range(NFC):
                            nc.tensor.matmul(
                                ps_y[:], w_o[:, fc, ts(mo, P)],
                                sdpaT[:, fc, ts(c, SC)],
                                start=(fc == 0), stop=(fc == NFC - 1),
                                skip_group_check=True,
                            )
                        ys = ytile.tile([P, SC], bf16, tag="y",
                                        name=f"ys_{c}_{mo}")
                        nc.vector.tensor_copy(ys[:], ps_y[:])
                        nc.sync.dma_start(yT_r[:, mo, ts(c, SC)], ys[:])
                    units.append(uw)
                return units

            # ---- attention for one chunk with interleaved fillers ----
            def attn(c, fillers):
                n_j = 4 * c + 4
                slots = 2 * n_j
                fidx = 0
                slot = 0
                for hp in range(2):
                    h0, h1 = 2 * hp, 2 * hp + 1
                    ps_o = {}
                    for h in (h0, h1):
                        ps_o[h] = pso.tile(
                            [DK + 1, SC], f32, tag="po", name=f"po_{c}_{h}"
                        )
                    j_order = list(range(4 * c, n_j)) + list(range(4 * c))
                    prev = None
                    for jx, j in enumerate(j_order):
                        ps2 = pss.tile([P, 2, SC], f32, tag="s2",
                                       name=f"s2_{c}_{hp}_{jx}")
                        nc.tensor.matmul(
                            ps2[:, 0, :], kT[0:DK, hp, ts(j, P)],
                            qT[0:DK, hp, ts(c, SC)],
                            start=True, stop=True, skip_group_check=True,
                        )
                        nc.tensor.matmul(
                            ps2[:, 1, :], kT[DK : 2 * DK, hp, ts(j, P)],
                            qT[DK : 2 * DK, hp, ts(c, SC)],
                            start=True, stop=True, skip_group_check=True,
                        )
                        et2 = etp.tile([P, 2, SC], bf16, tag="et",
                                       name=f"et_{c}_{hp}_{jx}")
                        d = j - 4 * c
                        if d >= 0:
                            off = P * d
                            if off:
                                nc.gpsimd.memset(et2[:, :, :off], 0.0)
                                nc.scalar.activation(
                                    et2[:, :, off:], ps2[:, :, off:], AF.Exp
                                )
                            else:
                                nc.scalar.activation(et2[:], ps2[:], AF.Exp)
                            nc.gpsimd.tensor_tensor(
                                et2[:, :, off : off + P], et2[:, :, off : off + P],
                                masks[:], bass.mybir.AluOpType.mult,
                            )
                        else:
                            nc.scalar.activation(et2[:], ps2[:], AF.Exp)
                        if prev is not None:
                            pj, pet = prev
                            for h in (h0, h1):
                                nc.tensor.matmul(
                                    ps_o[h][:],
                                    vaug[:, pj, (DK + 1) * h : (DK + 1) * (h + 1)],
                                    pet[:, h % 2, :],
                                    start=(jx == 1), stop=False,
                                    skip_group_check=True,
                                )
                        prev = (j, et2)
                        slot += 1
                        want = (len(fillers) * slot) // slots
                        while fidx < want:
                            fillers[fidx]()
                            fidx += 1
                    pj, pet = prev
                    for h in (h0, h1):
                        nc.tensor.matmul(
                            ps_o[h][:],
                            vaug[:, pj, (DK + 1) * h : (DK + 1) * (h + 1)],
                            pet[:, h % 2, :],
                            start=(n_j == 1), stop=True,
                            skip_group_check=True,
                        )
                    # softmax normalize: denom -> broadcast -> recip -> mult
                    for h in (h0, h1):
                        dbf = small.tile([1, SC], bf16, tag="dbf",
                                         name=f"dbf_{c}_{h}")
                        nc.vector.tensor_copy(dbf[:], ps_o[h][DK : DK + 1, :])
                        ps_bc = pp.tile([P, SC], f32, tag="p512",
                                        name=f"bc_{c}_{h}")
                        nc.tensor.matmul(
                            ps_bc[:DK, :], ones_r[:], dbf[:],
                            start=True, stop=True, skip_group_check=True,
                        )
                        rc64 = small.tile([DK, SC], f32, tag="rc",
                                          name=f"rc_{c}_{h}")
                        nc.vector.reciprocal_approx_fast(rc64[:], ps_bc[:DK, :])
                        base = DK * (h % 2)
                        nc.vector.tensor_tensor(
                            sdpaT[base : base + DK, h // 2, ts(c, SC)],
                            ps_o[h][:DK, :], rc64[:],
                            bass.mybir.AluOpType.mult,
                        )
                while fidx < len(fillers):
                    fillers[fidx]()
                    fidx += 1

            # ---- main pipeline ----
            for u in make_proj_fillers(0):
                u()
            for c in range(NSC):
                fillers = []
                if c + 1 < NSC:
                    fillers += make_proj_fillers(c + 1)
                if c >= 1:
                    fillers += make_wo_fillers(c - 1)
                attn(c, fillers)
            for u in make_wo_fillers(NSC - 1):
                u()

    nc.compile()
    return nc


def _get_nc():
    if "nc" not in _state:
        _state["nc"] = _build_nc()
    return _state["nc"]


def kernel(Q, K, V, mask, Wq, bq, Wk, bk, Wv, bv, Wo, bo):
    global last_results
    from concourse.bass_utils import run_bass_kernel_spmd
    import ml_dtypes

    bf16 = ml_dtypes.bfloat16

    Q = np.asarray(Q, np.float32)
    K = np.asarray(K, np.float32)
    V = np.asarray(V, np.float32)
    Wq = np.asarray(Wq, np.float32)
    bq = np.asarray(bq, np.float32)
    Wk = np.asarray(Wk, np.float32)
    bk = np.asarray(bk, np.float32)
    Wv = np.asarray(Wv, np.float32)
    bv = np.asarray(bv, np.float32)
    Wo = np.asarray(Wo, np.float32)
    bo = np.asarray(bo, np.float32)

    nc = _get_nc()

    # causal triangle mask block: keep[p, x] = (x >= p), dup for 2 heads
    p = np.arange(P)[:, None]
    x = np.arange(P)[None, :]
    tri = (x >= p).astype(bf16)                       # [P, P]
    msk_np = np.ascontiguousarray(
        np.broadcast_to(tri[:, None, :], (P, 2, P))
    )

    xT = {}
    for b in range(B):
        xT[("q", b)] = np.ascontiguousarray(Q[b].T.astype(bf16))
        xT[("k", b)] = np.ascontiguousarray(K[b].T.astype(bf16))
        xT[("v", b)] = np.ascontiguousarray(V[b].T.astype(bf16))

    in_maps = []
    for core in range(8):
        b = core // 4
        g = core % 4
        fs, fe = FL * g, FL * (g + 1)
        # fold the 1/sqrt(dk)=0.125 score scale into the q side (exact)
        wq_s = np.ascontiguousarray((Wq[fs:fe, :] * 0.125).T.astype(bf16))
        bq_s = bq[fs:fe] * 0.125
        in_maps.append({
            "xqT": xT[("q", b)],
            "xkT": xT[("k", b)],
            "xvT": xT[("v", b)],
            "wqT": wq_s,
            "wkT": np.ascontiguousarray(Wk[fs:fe, :].T.astype(bf16)),
            "wvT": np.ascontiguousarray(Wv[fs:fe, :].T.astype(bf16)),
            "woT": np.ascontiguousarray(Wo[:, fs:fe].T.astype(bf16)),
            "bqs": np.ascontiguousarray(bq_s.reshape(NFC, P).T),
            "bks": np.ascontiguousarray(bk[fs:fe].reshape(NFC, P).T),
            "bvb": np.ascontiguousarray(
                np.broadcast_to(bv[fs:fe][None, :], (P, FL))
            ),
            "msk": msk_np,
        })

    res = run_bass_kernel_spmd(nc, in_maps, core_ids=list(range(8)))
    last_results = res

    out = np.empty((B, S, D), np.float32)
    for b in range(B):
        acc = res.results[4 * b]["yT"].astype(np.float32)
        for g in range(1, 4):
            acc = acc + res.results[4 * b + g]["yT"].astype(np.float32)
        out[b] = acc.T + bo[None, :]
    return out
